# revision 16
# baseline (speedup 1.0000x reference)
"""GAT layer kernel for Trainium2, 8 NeuronCores.

Sharding: 16 (b, h) pairs -> 8 cores. Core k handles batch b = k//2 and the
head pair hp = k%2 (heads 2*hp, 2*hp+1). adj (as an additive fp16 mask, host
pre-transposed) is replicated; each core runs the full N^2 attention for its
two heads, one head at a time: head 0's pair ReduceScatter runs on the
collective engines underneath head 1's hot loop (also warming the ncfw
control path), so only head 1's RS is exposed at the end. Each core emits
half the rows of the final output and the host concatenates.

Math per (b, h), softmax over the *i* axis (rows) of e[i, j]:
  h    = x[b] @ W[h]                         [N, F]
  f1_i = h_i . a1,  f2_j = h_j . a2
  v[j, i]  = f1_i + f2_j + M[j, i]           (M = 0 on edge, -150 masked)
  L        = max(v, 0.2*v)                   (= leaky, exp-monotone safe)
  Em[j, i] = exp(L)    ;  s_j = sum_i Em[j, i]   (ACT accum_out, fused)
  g[j, :]  = h[j, :] / s_j                   (gpsimd normalize_recip)
  hpT[f, i] = sum_j g[j, f] * Em[j, i]       (PE, transposed-out layout)
  out = leaky(relu(hp) cat-heads @ Wl.T + bl)

All projections run in fp16 on the PE. f1 is produced directly in broadcast
form (F1B) by a matmul against a host-replicated W@a1 block; f2 rides as a
65th output column of the h projection (W@a2 appended to W). The final
bias + pair-sum accumulates via CCE-add DMAs onto a bias-prefilled tile.
"""

import sys

import numpy as np

sys.path.insert(0, "/opt/trn_rl_repo")

from concourse import bacc, bass, dve_ops, mybir, tile  # noqa: E402
from concourse.bass_utils import run_bass_kernel_spmd  # noqa: E402
from concourse.dve_spec import C0, C1, C2, Spec, Src0, Src1, relu  # noqa: E402

# Fused leaky-relu of a masked outer sum, one DVE pass at 1x:
#   out = leaky(in0 + s0 + in1) = s1*v + imm2*relu(v),  v = in0 + s0 + in1
# (in0 = broadcast f1 row, s0 = per-partition f2, in1 = additive adj mask).
_v = (Src0 + C0) + Src1
LEAKY_MASK_ANT = dve_ops.DveOp(
    "LEAKY_MASK_ANT",
    Spec(
        body=_v * C1 + relu(_v) * C2,
        reference=lambda in0, in1, s0, s1, imm2: (
            lambda v: (v * s1 + np.maximum(v, 0) * imm2).astype(np.float32)
        )(in0.astype(np.float32) + s0 + in1),
    ),
    subdim=False,
    uops_sha={"v3": "61445124be53cf8e", "v4": "fd84e7f03d2c00e0"},
)
if LEAKY_MASK_ANT.name not in dve_ops._SUB_OPCODE_FOR_NAME:
    dve_ops.OPS.append(LEAKY_MASK_ANT)
    dve_ops._SUB_OPCODE_FOR_NAME[LEAKY_MASK_ANT.name] = (
        dve_ops._CUSTOM_DVE_ROW_BASE + len(dve_ops.OPS) - 1)
    dve_ops.CUSTOM_DVE_SPECS[LEAKY_MASK_ANT.name] = LEAKY_MASK_ANT.spec

B, N, C, F, H = 4, 2048, 256, 64, 4
P = 128
NT = N // P  # 16 j-tiles / n-chunks
HNT = NT // 2  # 8 n-chunks per core in the final output half
CT = C // P  # 2 contraction tiles over Cin
IC = 512  # i-chunk (matmul moving free dim / psum bank)
NIC = N // IC  # 4
FE = F + 1  # h projection emits f2 as a 65th column
HG = 4  # j-tiles per h-projection psum group (4*FE=260 fits one bank)
MADD_GROUPS = [2, 2, 4, 4, 4]  # j-tiles per madd DMA (small first)
# params blob free-dim layout (all sections 128-partition, fp16)
PRM_WE = 0                      # w_ext: [P, 2, CT, FE]  2*CT*FE = 260
PRM_WA1 = PRM_WE + 2 * CT * FE  # wa1r:  [P, 2, CT, P]   2*CT*P = 512
PRM_WLT = PRM_WA1 + 2 * CT * P  # wlT:   [P, F]          64
PRM_BLT = PRM_WLT + F           # blt:   [P, HNT*F]      512
PRM_LEN = PRM_BLT + HNT * F
ALPHA = 0.2
MASKV = 150.0  # additive mask magnitude; exp(0.2 * -150) ~ 1e-13
NCORES = 8

F32 = mybir.dt.float32
F16 = mybir.dt.float16
ADD = mybir.AluOpType.add
MULT = mybir.AluOpType.mult
MAX = mybir.AluOpType.max

_CACHE = {}


def _build_program():
    nc = bacc.Bacc("TRN2", target_bir_lowering=False, debug=False,
                   num_devices=NCORES)

    xT = nc.dram_tensor("xT", [C, N], F16, kind="ExternalInput")
    madd = nc.dram_tensor("madd", [N, N], F16, kind="ExternalInput")
    params = nc.dram_tensor("params", [P, PRM_LEN], F16, kind="ExternalInput")
    out = nc.dram_tensor("out", [N // 2, F], F32, kind="ExternalOutput")

    cc_in = [nc.dram_tensor(f"cc_in{hl}", [N, F], F16) for hl in range(2)]
    cc_out = [nc.dram_tensor(f"cc_out{hl}", [N // 2, F], F16)
              for hl in range(2)]

    with tile.TileContext(nc) as tc:
        with (
            tc.tile_pool(name="const", bufs=1) as const,
            tc.tile_pool(name="head", bufs=2) as head,
            tc.tile_pool(name="vm", bufs=3) as vm_pool,
            tc.tile_pool(name="em", bufs=3) as em_pool,
            tc.tile_pool(name="g", bufs=4) as g_pool,
            tc.tile_pool(name="psA", bufs=3, space="PSUM") as psA,
            tc.tile_pool(name="psB", bufs=1, space="PSUM") as psB,
        ):
            # ---- constant loads -------------------------------------------
            prm_sb = const.tile([P, PRM_LEN], F16)
            nc.sync.dma_start(prm_sb[:], params[:])
            we_sb = prm_sb[:, PRM_WE:PRM_WA1].rearrange(
                "p (h c f) -> p h c f", h=2, c=CT)
            wa1_sb = prm_sb[:, PRM_WA1:PRM_WLT].rearrange(
                "p (h c q) -> p h c q", h=2, c=CT)
            wlT_sb = prm_sb[:, PRM_WLT:PRM_BLT]
            blt_sb = prm_sb[:, PRM_BLT:PRM_LEN]
            # xT in i-chunks so phase A starts under the DMA
            xT_sb = const.tile([P, CT, N], F16)
            for icc in range(NIC):
                nc.sync.dma_start(
                    xT_sb[:, :, icc * IC:(icc + 1) * IC],
                    xT.rearrange("(c p) n -> p c n", p=P)[
                        :, :, icc * IC:(icc + 1) * IC])
            # madd in groups; first groups small so the hot loop starts early
            madd_sb = []
            row0 = 0
            for gi, gn in enumerate(MADD_GROUPS):
                t = const.tile([P, gn, N], F16, tag=f"madd{gi}",
                               name=f"madd_sb{gi}")
                nc.sync.dma_start(
                    t[:],
                    madd[row0 * P:(row0 + gn) * P, :].rearrange(
                        "(t p) n -> p t n", p=P))
                for k in range(gn):
                    madd_sb.append((t, k))
                row0 += gn

            # gpsimd ext-isa warmup: absorb the ~6us attn-library IRAM load
            # during the input DMAs instead of the first hot-loop iteration
            wrm_sb = const.tile([P, 3], F32)
            nc.gpsimd.memset(wrm_sb[:], 1.0)
            nc.gpsimd.normalize_recip(wrm_sb[:, 2:3], wrm_sb[:, 1:2],
                                      wrm_sb[:, 0:1])

            # pair-sum target, pre-filled with the bias; both heads' RS
            # results land on it via CCE-add DMAs
            ys_sb = const.tile([P, HNT * F], F16)
            nc.any.tensor_copy(ys_sb[:], blt_sb)

            catT_sb = const.tile([P, N], F16)
            part_sb = [const.tile([P, NT * F], F16, tag=f"part{hl}",
                                  name=f"part_sb{hl}") for hl in range(2)]

            for hl in range(2):
                # ---- phase A: fp16 projections ---------------------------
                F1B_sb = head.tile([P, N], F16, tag="F1B")
                h_sb = head.tile([P, NT, F], F32, tag="h")
                f2c_sb = head.tile([P, NT], F32, tag="f2c")
                sc_sb = head.tile([P, NT], F32, tag="sc")

                # F1B[j, i] = f1[i] = sum_c wa1[c] * xT[c, i] (j-broadcast
                # via host-replicated wa1 columns)
                for icc in range(NIC):
                    ps = psA.tile([P, IC], F32, tag="psA")
                    for ct in range(CT):
                        nc.tensor.matmul(
                            ps[:], wa1_sb[:, hl, ct, :],
                            xT_sb[:, ct, icc * IC:(icc + 1) * IC],
                            start=(ct == 0), stop=(ct == CT - 1))
                    nc.any.tensor_copy(F1B_sb[:, icc * IC:(icc + 1) * IC],
                                       ps[:])
                # h[n, 0:64] and f2[n] (col 64) in one projection against
                # w_ext = [W | W@a2]
                for grp in range(NT // HG):
                    ps = psA.tile([P, IC], F32, tag="psA")
                    for k in range(HG):
                        jt = grp * HG + k
                        for ct in range(CT):
                            nc.tensor.matmul(
                                ps[:, k * FE:(k + 1) * FE],
                                xT_sb[:, ct, jt * P:(jt + 1) * P],
                                we_sb[:, hl, ct, :],
                                start=(ct == 0), stop=(ct == CT - 1))
                    psv = ps[:, 0:HG * FE].rearrange(
                        "p (k f) -> p k f", f=FE)
                    nc.any.tensor_copy(
                        h_sb[:, grp * HG:(grp + 1) * HG, :],
                        psv[:, :, 0:F])
                    nc.any.tensor_copy(
                        f2c_sb[:, grp * HG:(grp + 1) * HG],
                        psv[:, :, F:FE].rearrange("p k o -> p (k o)"))

                # ---- hot loop: masked exp-leaky attention ---------------
                hpT = psB.tile([P, N], F32, tag="hpT")
                for jt in range(NT):
                    mt, mk = madd_sb[jt]
                    lk = vm_pool.tile([P, N], F16, tag="lk")
                    nc.vector._custom_dve(
                        LEAKY_MASK_ANT, out=lk[:], in0=F1B_sb[:],
                        in1=mt[:, mk, :],
                        s0=f2c_sb[:, jt:jt + 1],
                        s1=float(ALPHA), imm2=1.0 - ALPHA)
                    em = em_pool.tile([P, N], F16, tag="em")
                    nc.scalar.activation(
                        em[:], lk[:], mybir.ActivationFunctionType.Exp,
                        accum_out=sc_sb[:, jt:jt + 1])
                    # g = h / s on the (otherwise idle) gpsimd engine
                    g = g_pool.tile([P, F], F16, tag="g")
                    nc.gpsimd.normalize_recip(g[:], h_sb[:, jt, :],
                                              sc_sb[:, jt:jt + 1])
                    for icc in range(NIC):
                        nc.tensor.matmul(
                            hpT[hl * F:(hl + 1) * F,
                                icc * IC:(icc + 1) * IC],
                            g[:], em[:, icc * IC:(icc + 1) * IC],
                            start=(jt == 0), stop=(jt == NT - 1))
                # relu(hp) into the concat-head tile (same partitions)
                nc.scalar.activation(catT_sb[hl * F:(hl + 1) * F, :],
                                     hpT[hl * F:(hl + 1) * F, :],
                                     mybir.ActivationFunctionType.Relu)

                # ---- per-head final-linear partial + pair ReduceScatter.
                # Head 0's RS runs while head 1's hot loop computes.
                for grp in range(2):
                    ps = psA.tile([P, IC], F32, tag="psA")
                    for k in range(8):
                        ncu = grp * 8 + k
                        nc.tensor.matmul(
                            ps[:, k * F:(k + 1) * F],
                            catT_sb[hl * F:(hl + 1) * F,
                                    ncu * P:(ncu + 1) * P],
                            wlT_sb[hl * F:(hl + 1) * F, :],
                            start=True, stop=True)
                    nc.any.tensor_copy(
                        part_sb[hl][:, grp * IC:(grp + 1) * IC], ps[:])
                nc.sync.dma_start(
                    cc_in[hl].rearrange("(c p) f -> p c f", p=P),
                    part_sb[hl][:].rearrange("p (c f) -> p c f", f=F))
                nc.gpsimd.collective_compute(
                    "ReduceScatter", ADD,
                    replica_groups=[[0, 1], [2, 3], [4, 5], [6, 7]],
                    ins=[cc_in[hl][:]], outs=[cc_out[hl][:]])

            # ---- tail: ys = blt + RS(head0) + RS(head1) via CCE-add DMAs,
            # then one leaky pass and the output store
            for hl in range(2):
                nc.gpsimd.dma_start(
                    ys_sb[:].rearrange("p (c f) -> p c f", f=F),
                    cc_out[hl].rearrange("(c p) f -> p c f", p=P),
                    accum_op=ADD)
            yo_sb = const.tile([P, HNT * F], F32)
            nc.vector.scalar_tensor_tensor(
                yo_sb[:], ys_sb[:], float(ALPHA), ys_sb[:],
                op0=MULT, op1=MAX)
            nc.sync.dma_start(
                out.rearrange("(c p) f -> p c f", p=P),
                yo_sb[:].rearrange("p (c f) -> p c f", f=F))

    nc.compile()
    return nc


def get_program():
    if "nc" not in _CACHE:
        _CACHE["nc"] = _build_program()
    return _CACHE["nc"]


def make_in_maps(x, adj, W, a1, a2, Wl, bl):
    x = np.asarray(x, dtype=np.float32)
    adj = np.asarray(adj)
    W = np.asarray(W, dtype=np.float32)
    a1 = np.asarray(a1, dtype=np.float32)
    a2 = np.asarray(a2, dtype=np.float32)
    Wl = np.asarray(Wl, dtype=np.float32)
    bl = np.asarray(bl, dtype=np.float32)

    madd = ((MASKV * adj.T.astype(np.float32)) - MASKV).astype(np.float16)
    madd = np.ascontiguousarray(madd)
    WlT = np.ascontiguousarray(Wl.T)  # [H*F, F]
    blt = np.tile(bl, (P, HNT)).astype(np.float16)
    wa1 = np.einsum("hcf,hf->hc", W, a1)  # [H, C]
    wa2 = np.einsum("hcf,hf->hc", W, a2)  # [H, C]
    w_ext = np.concatenate([W, wa2[:, :, None]], axis=2)  # [H, C, FE]

    in_maps = []
    for k in range(NCORES):
        b, hp = k // 2, k % 2
        hs = slice(2 * hp, 2 * hp + 2)
        # params blob: [P, PRM_LEN] fp16; section layouts match the sbuf
        # views ("p (h c f) -> p h c f" etc. with c-chunked partition dim)
        prm = np.empty((P, PRM_LEN), np.float16)
        prm[:, PRM_WE:PRM_WA1] = w_ext[hs].reshape(2, CT, P, FE).transpose(
            2, 0, 1, 3).reshape(P, -1).astype(np.float16)
        wa1r = np.broadcast_to(wa1[hs].reshape(2, CT, P, 1),
                               (2, CT, P, P))
        prm[:, PRM_WA1:PRM_WLT] = wa1r.transpose(2, 0, 1, 3).reshape(
            P, -1).astype(np.float16)
        prm[:, PRM_WLT:PRM_BLT] = WlT[hp * P:(hp + 1) * P].astype(np.float16)
        prm[:, PRM_BLT:PRM_LEN] = blt
        in_maps.append({
            "xT": np.ascontiguousarray(x[b].T).astype(np.float16),
            "madd": madd,
            "params": prm,
        })
    return in_maps


def assemble_out(per_core_out):
    """per_core_out[k] = [N//2, F] half owned by core k; returns [B, N, F]."""
    out = np.empty((B, N, F), dtype=np.float32)
    for b in range(B):
        out[b, :N // 2] = per_core_out[2 * b]
        out[b, N // 2:] = per_core_out[2 * b + 1]
    return out


def kernel(x, adj, W, a1, a2, Wl, bl, _results=None, **run_kwargs):
    nc = get_program()
    in_maps = make_in_maps(x, adj, W, a1, a2, Wl, bl)
    res = run_bass_kernel_spmd(nc, in_maps, core_ids=list(range(NCORES)),
                               **run_kwargs)
    if _results is not None:
        _results.append(res)
    return assemble_out([res.results[k]["out"] for k in range(NCORES)])


# revision 22
# speedup vs baseline: 1.2723x; 1.2723x over previous
"""GAT layer kernel for Trainium2, 8 NeuronCores.

Sharding: 16 (b, h) pairs -> 8 cores. Core k handles batch b = k//2 and the
head pair hp = k%2 (heads 2*hp, 2*hp+1). adj (as an additive fp16 mask, host
pre-transposed) is replicated; each core runs the full N^2 attention for its
two heads, one head at a time: head 0's pair ReduceScatter runs on the
collective engines underneath head 1's hot loop (also warming the ncfw
control path), so only head 1's RS is exposed at the end. Each core emits
half the rows of the final output and the host concatenates.

Math per (b, h), softmax over the *i* axis (rows) of e[i, j]:
  h    = x[b] @ W[h]                         [N, F]
  f1_i = h_i . a1,  f2_j = h_j . a2
  v[j, i]  = f1_i + f2_j + M[j, i]           (M = 0 on edge, -150 masked)
  L        = max(v, 0.2*v)                   (= leaky, exp-monotone safe)
  Em[j, i] = exp(L)    ;  s_j = sum_i Em[j, i]   (ACT accum_out, fused)
  g[j, :]  = h[j, :] / s_j
  hpT[f, i] = sum_j g[j, f] * Em[j, i]       (PE, transposed-out layout)
  out = leaky(relu(hp) cat-heads @ Wl.T + bl)

All projections run in fp16 on the PE (fp32 matmul double-pumps the array as
LOW/HIGH passes). f1 is produced directly in broadcast form (F1B) by a
matmul against a host-replicated W@a1 block; f2 rides as a 65th output
column of the h projection (W@a2 appended to W).
"""

import sys

import numpy as np

sys.path.insert(0, "/opt/trn_rl_repo")

from concourse import bacc, bass, dve_ops, mybir, tile  # noqa: E402
from concourse.bass_utils import run_bass_kernel_spmd  # noqa: E402
from concourse.dve_spec import C0, C1, C2, Spec, Src0, Src1, relu  # noqa: E402

# Fused leaky-relu of a masked outer sum, one DVE pass at 1x:
#   out = leaky(in0 + s0 + in1) = s1*v + imm2*relu(v),  v = in0 + s0 + in1
# (in0 = broadcast f1 row, s0 = per-partition f2, in1 = additive adj mask).
_v = (Src0 + C0) + Src1
LEAKY_MASK_ANT = dve_ops.DveOp(
    "LEAKY_MASK_ANT",
    Spec(
        body=_v * C1 + relu(_v) * C2,
        reference=lambda in0, in1, s0, s1, imm2: (
            lambda v: (v * s1 + np.maximum(v, 0) * imm2).astype(np.float32)
        )(in0.astype(np.float32) + s0 + in1),
    ),
    subdim=False,
    uops_sha={"v3": "61445124be53cf8e", "v4": "fd84e7f03d2c00e0"},
)
if LEAKY_MASK_ANT.name not in dve_ops._SUB_OPCODE_FOR_NAME:
    dve_ops.OPS.append(LEAKY_MASK_ANT)
    dve_ops._SUB_OPCODE_FOR_NAME[LEAKY_MASK_ANT.name] = (
        dve_ops._CUSTOM_DVE_ROW_BASE + len(dve_ops.OPS) - 1)
    dve_ops.CUSTOM_DVE_SPECS[LEAKY_MASK_ANT.name] = LEAKY_MASK_ANT.spec

B, N, C, F, H = 4, 2048, 256, 64, 4
P = 128
NT = N // P  # 16 j-tiles / n-chunks
HNT = NT // 2  # 8 n-chunks per core in the final output half
CT = C // P  # 2 contraction tiles over Cin
IC = 512  # i-chunk (matmul moving free dim / psum bank)
NIC = N // IC  # 4
FE = F + 1  # h projection emits f2 as a 65th column
HG = 4  # j-tiles per h-projection psum group (4*FE=260 fits one bank)
MADD_GROUPS = [2, 2, 4, 4, 4]  # j-tiles per madd DMA (small first)
# params blob free-dim layout (all sections 128-partition, fp16)
PRM_WE = 0                      # w_ext: [P, 2, CT, FE]  2*CT*FE = 260
PRM_WA1 = PRM_WE + 2 * CT * FE  # wa1r:  [P, 2, CT, P]   2*CT*P = 512
PRM_WLT = PRM_WA1 + 2 * CT * P  # wlT:   [P, F]          64
PRM_BLT = PRM_WLT + F           # blt:   [P, HNT*F]      512
PRM_LEN = PRM_BLT + HNT * F
ALPHA = 0.2
MASKV = 150.0  # additive mask magnitude; exp(0.2 * -150) ~ 1e-13
NCORES = 8

F32 = mybir.dt.float32
F16 = mybir.dt.float16
ADD = mybir.AluOpType.add
MULT = mybir.AluOpType.mult
MAX = mybir.AluOpType.max

_CACHE = {}


def _build_program():
    nc = bacc.Bacc("TRN2", target_bir_lowering=False, debug=False,
                   num_devices=NCORES)

    xT = nc.dram_tensor("xT", [C, N], F16, kind="ExternalInput")
    madd = nc.dram_tensor("madd", [N, N], F16, kind="ExternalInput")
    params = nc.dram_tensor("params", [P, PRM_LEN], F16, kind="ExternalInput")
    out = nc.dram_tensor("out", [N // 2, F], F32, kind="ExternalOutput")

    cc_in = [nc.dram_tensor(f"cc_in{hl}", [N, F], F16) for hl in range(2)]
    cc_out = [nc.dram_tensor(f"cc_out{hl}", [N // 2, F], F16)
              for hl in range(2)]

    with tile.TileContext(nc) as tc:
        with (
            tc.tile_pool(name="const", bufs=1) as const,
            tc.tile_pool(name="head", bufs=2) as head,
            tc.tile_pool(name="vm", bufs=3) as vm_pool,
            tc.tile_pool(name="em", bufs=3) as em_pool,
            tc.tile_pool(name="g", bufs=4) as g_pool,
            tc.tile_pool(name="psF", bufs=1, space="PSUM") as psF,
            tc.tile_pool(name="psH", bufs=1, space="PSUM") as psH,
            tc.tile_pool(name="psFin", bufs=1, space="PSUM") as psFin,
            tc.tile_pool(name="psB", bufs=1, space="PSUM") as psB,
        ):
            # ---- constant loads -------------------------------------------
            prm_sb = const.tile([P, PRM_LEN], F16)
            nc.sync.dma_start(prm_sb[:], params[:])
            we_sb = prm_sb[:, PRM_WE:PRM_WA1].rearrange(
                "p (h c f) -> p h c f", h=2, c=CT)
            wa1_sb = prm_sb[:, PRM_WA1:PRM_WLT].rearrange(
                "p (h c q) -> p h c q", h=2, c=CT)
            wlT_sb = prm_sb[:, PRM_WLT:PRM_BLT]
            blt_sb = prm_sb[:, PRM_BLT:PRM_LEN]
            # xT in i-chunks so phase A starts under the DMA
            xT_sb = const.tile([P, CT, N], F16)
            for icc in range(NIC):
                nc.sync.dma_start(
                    xT_sb[:, :, icc * IC:(icc + 1) * IC],
                    xT.rearrange("(c p) n -> p c n", p=P)[
                        :, :, icc * IC:(icc + 1) * IC])
            # madd in groups; first groups small so the hot loop starts early
            madd_sb = []
            row0 = 0
            for gi, gn in enumerate(MADD_GROUPS):
                t = const.tile([P, gn, N], F16, tag=f"madd{gi}",
                               name=f"madd_sb{gi}")
                nc.sync.dma_start(
                    t[:],
                    madd[row0 * P:(row0 + gn) * P, :].rearrange(
                        "(t p) n -> p t n", p=P))
                for k in range(gn):
                    madd_sb.append((t, k))
                row0 += gn

            catT_sb = const.tile([P, N], F16)
            part_sb = [const.tile([P, NT * F], F16, tag=f"part{hl}",
                                  name=f"part_sb{hl}") for hl in range(2)]

            for hl in range(2):
                # ---- phase A: fp16 projections ---------------------------
                F1B_sb = head.tile([P, N], F16, tag="F1B")
                h_sb = head.tile([P, NT, F], F16, tag="h")
                f2c_sb = head.tile([P, NT], F32, tag="f2c")
                sc_sb = head.tile([P, NT], F32, tag="sc")
                rc_sb = head.tile([P, NT], F32, tag="rc")

                # F1B[j, i] = f1[i] = sum_c wa1[c] * xT[c, i] (j-broadcast
                # via host-replicated wa1 columns)
                for icc in range(NIC):
                    ps = psF.tile([P, IC], F32, tag="psF")
                    for ct in range(CT):
                        nc.tensor.matmul(
                            ps[:], wa1_sb[:, hl, ct, :],
                            xT_sb[:, ct, icc * IC:(icc + 1) * IC],
                            start=(ct == 0), stop=(ct == CT - 1))
                    nc.any.tensor_copy(F1B_sb[:, icc * IC:(icc + 1) * IC],
                                       ps[:])
                # h[n, 0:64] and f2[n] (col 64) in one projection against
                # w_ext = [W | W@a2]
                for grp in range(NT // HG):
                    ps = psH.tile([P, IC], F32, tag="psH")
                    for k in range(HG):
                        jt = grp * HG + k
                        for ct in range(CT):
                            nc.tensor.matmul(
                                ps[:, k * FE:(k + 1) * FE],
                                xT_sb[:, ct, jt * P:(jt + 1) * P],
                                we_sb[:, hl, ct, :],
                                start=(ct == 0), stop=(ct == CT - 1))
                    psv = ps[:, 0:HG * FE].rearrange(
                        "p (k f) -> p k f", f=FE)
                    nc.any.tensor_copy(
                        h_sb[:, grp * HG:(grp + 1) * HG, :],
                        psv[:, :, 0:F])
                    nc.any.tensor_copy(
                        f2c_sb[:, grp * HG:(grp + 1) * HG],
                        psv[:, :, F:FE].rearrange("p k o -> p (k o)"))

                # ---- hot loop: masked exp-leaky attention ---------------
                hpT = psB.tile([P, N], F32, tag="hpT")
                for jt in range(NT):
                    mt, mk = madd_sb[jt]
                    lk = vm_pool.tile([P, N], F16, tag="lk")
                    nc.vector._custom_dve(
                        LEAKY_MASK_ANT, out=lk[:], in0=F1B_sb[:],
                        in1=mt[:, mk, :],
                        s0=f2c_sb[:, jt:jt + 1],
                        s1=float(ALPHA), imm2=1.0 - ALPHA)
                    em = em_pool.tile([P, N], F16, tag="em")
                    nc.scalar.activation(
                        em[:], lk[:], mybir.ActivationFunctionType.Exp,
                        accum_out=sc_sb[:, jt:jt + 1])
                    nc.vector.reciprocal(rc_sb[:, jt:jt + 1],
                                         sc_sb[:, jt:jt + 1])
                    g = g_pool.tile([P, F], F16, tag="g")
                    nc.vector.tensor_scalar_mul(g[:], h_sb[:, jt, :],
                                                rc_sb[:, jt:jt + 1])
                    for icc in range(NIC):
                        nc.tensor.matmul(
                            hpT[hl * F:(hl + 1) * F,
                                icc * IC:(icc + 1) * IC],
                            g[:], em[:, icc * IC:(icc + 1) * IC],
                            start=(jt == 0), stop=(jt == NT - 1))
                # relu(hp) into the concat-head tile (same partitions)
                nc.scalar.activation(catT_sb[hl * F:(hl + 1) * F, :],
                                     hpT[hl * F:(hl + 1) * F, :],
                                     mybir.ActivationFunctionType.Relu)

                # ---- per-head final-linear partial + pair ReduceScatter.
                # Head 0's RS runs while head 1's hot loop computes.
                for grp in range(2):
                    ps = psFin.tile([P, IC], F32, tag="psFin")
                    for k in range(8):
                        ncu = grp * 8 + k
                        nc.tensor.matmul(
                            ps[:, k * F:(k + 1) * F],
                            catT_sb[hl * F:(hl + 1) * F,
                                    ncu * P:(ncu + 1) * P],
                            wlT_sb[hl * F:(hl + 1) * F, :],
                            start=True, stop=True)
                    nc.any.tensor_copy(
                        part_sb[hl][:, grp * IC:(grp + 1) * IC], ps[:])
                nc.sync.dma_start(
                    cc_in[hl].rearrange("(c p) f -> p c f", p=P),
                    part_sb[hl][:].rearrange("p (c f) -> p c f", f=F))
                nc.gpsimd.collective_compute(
                    "ReduceScatter", ADD,
                    replica_groups=[[0, 1], [2, 3], [4, 5], [6, 7]],
                    ins=[cc_in[hl][:]], outs=[cc_out[hl][:]])

            # ---- tail: combine the two heads' RS halves + bias + leaky
            ys_sb = [const.tile([P, HNT * F], F16, tag=f"ys{hl}",
                                name=f"ys_sb{hl}") for hl in range(2)]
            for hl in range(2):
                nc.sync.dma_start(
                    ys_sb[hl][:].rearrange("p (c f) -> p c f", f=F),
                    cc_out[hl].rearrange("(c p) f -> p c f", p=P))
            t1_sb = const.tile([P, HNT * F], F16)
            nc.vector.tensor_tensor(t1_sb[:], ys_sb[0][:], blt_sb, op=ADD)
            t2_sb = const.tile([P, HNT * F], F16)
            nc.vector.tensor_tensor(t2_sb[:], t1_sb[:], ys_sb[1][:], op=ADD)
            yo_sb = const.tile([P, HNT * F], F32)
            nc.vector.scalar_tensor_tensor(
                yo_sb[:], t2_sb[:], float(ALPHA), t2_sb[:],
                op0=MULT, op1=MAX)
            nc.sync.dma_start(
                out.rearrange("(c p) f -> p c f", p=P),
                yo_sb[:].rearrange("p (c f) -> p c f", f=F))

    nc.compile()
    return nc


def get_program():
    if "nc" not in _CACHE:
        _CACHE["nc"] = _build_program()
    return _CACHE["nc"]


def make_in_maps(x, adj, W, a1, a2, Wl, bl):
    x = np.asarray(x, dtype=np.float32)
    adj = np.asarray(adj)
    W = np.asarray(W, dtype=np.float32)
    a1 = np.asarray(a1, dtype=np.float32)
    a2 = np.asarray(a2, dtype=np.float32)
    Wl = np.asarray(Wl, dtype=np.float32)
    bl = np.asarray(bl, dtype=np.float32)

    madd = ((MASKV * adj.T.astype(np.float32)) - MASKV).astype(np.float16)
    madd = np.ascontiguousarray(madd)
    WlT = np.ascontiguousarray(Wl.T)  # [H*F, F]
    blt = np.tile(bl, (P, HNT)).astype(np.float16)
    wa1 = np.einsum("hcf,hf->hc", W, a1)  # [H, C]
    wa2 = np.einsum("hcf,hf->hc", W, a2)  # [H, C]
    w_ext = np.concatenate([W, wa2[:, :, None]], axis=2)  # [H, C, FE]

    in_maps = []
    for k in range(NCORES):
        b, hp = k // 2, k % 2
        hs = slice(2 * hp, 2 * hp + 2)
        # params blob: [P, PRM_LEN] fp16; section layouts match the sbuf
        # views ("p (h c f) -> p h c f" etc. with c-chunked partition dim)
        prm = np.empty((P, PRM_LEN), np.float16)
        prm[:, PRM_WE:PRM_WA1] = w_ext[hs].reshape(2, CT, P, FE).transpose(
            2, 0, 1, 3).reshape(P, -1).astype(np.float16)
        wa1r = np.broadcast_to(wa1[hs].reshape(2, CT, P, 1),
                               (2, CT, P, P))
        prm[:, PRM_WA1:PRM_WLT] = wa1r.transpose(2, 0, 1, 3).reshape(
            P, -1).astype(np.float16)
        prm[:, PRM_WLT:PRM_BLT] = WlT[hp * P:(hp + 1) * P].astype(np.float16)
        prm[:, PRM_BLT:PRM_LEN] = blt
        in_maps.append({
            "xT": np.ascontiguousarray(x[b].T).astype(np.float16),
            "madd": madd,
            "params": prm,
        })
    return in_maps


def assemble_out(per_core_out):
    """per_core_out[k] = [N//2, F] half owned by core k; returns [B, N, F]."""
    out = np.empty((B, N, F), dtype=np.float32)
    for b in range(B):
        out[b, :N // 2] = per_core_out[2 * b]
        out[b, N // 2:] = per_core_out[2 * b + 1]
    return out


def kernel(x, adj, W, a1, a2, Wl, bl, _results=None, **run_kwargs):
    nc = get_program()
    in_maps = make_in_maps(x, adj, W, a1, a2, Wl, bl)
    res = run_bass_kernel_spmd(nc, in_maps, core_ids=list(range(NCORES)),
                               **run_kwargs)
    if _results is not None:
        _results.append(res)
    return assemble_out([res.results[k]["out"] for k in range(NCORES)])
